# revision 1
# baseline (speedup 1.0000x reference)
"""Trainium2 Bass kernel: batched 64-digit base-10 addition (nn_Adder).

The reference RNN scan is just carry-propagating decimal addition:
    s_e = a_e + b_e; v_e = s_e + c_e; c_{e+1} = [v_e >= 10];
    digit_e = v_e mod 10   (digits stored MSB-first, carries run LSB->MSB)

Mapping onto one NeuronCore (pure data parallel across 8 cores, batch
524288 -> 65536 rows/core):

  * G=32 rows are packed per SBUF partition along the free dim, with a
    zero separator column before each 64-digit group. At a separator the
    scan state is the previous group's carry (0 or 1) < 10, so the carry
    into the next group's LSB is 0 -> ONE tensor_tensor_scan instruction
    carries 128*G rows.
  * s = a + b is computed on the idle TensorEngine as two accumulating
    identity matmuls into PSUM (psum = I@a, psum += I@b), 512-column
    chunks (one PSUM bank each).
  * ACT (ScalarEngine, own SBUF/PSUM ports) drains PSUM into the
    LSB-first separator layout (the MSB<->LSB reversal is folded into
    its access pattern).
  * DVE (VectorEngine) runs the carry scan
        v_t = [10 <= v_{t-1}] + s_t      (op0=is_le, op1=add)
    then c = [v >= 10] (tensor_scalar, 2x mode) and
    digit = c*(-10) + v (scalar_tensor_tensor) written back MSB-first
    via a reversed access pattern.
  * GpSimd is deliberately unused: any GpSimd op grabs the DVE shared
    SBUF port pair and degrades concurrent DVE ops ~3x.

All values are small integers, exact in f32 -> bit-exact output.
"""

import sys

sys.path.insert(0, "/opt/trn_rl_repo")

import numpy as np

BATCH = 524288
SEQ = 64
N_CORES = 8
B_LOC = BATCH // N_CORES

P = 128
GS = SEQ + 1        # group stride in s/w tiles (64 digits + 1 separator)
# per-tile digit-rows-per-partition schedule: small tiles at both ends
# shorten pipeline fill and the end-of-kernel drain
G_LIST = [8, 8, 16] + [32] * 14 + [16, 8, 8]
G_MAX = max(G_LIST)
IO_BUFS = 4
WK_BUFS = 3
N_SPP = 3           # ping-pong buffers for the separator-layout s tile
MMN = 512           # matmul free dim (one PSUM bank)

_nc_cache = {}


def _build_adder():
    from contextlib import ExitStack

    import concourse.bacc as bacc
    import concourse.bass as bass
    import concourse.mybir as mybir
    import concourse.tile as tile

    F32 = mybir.dt.float32
    ALU = mybir.AluOpType
    ACTF = mybir.ActivationFunctionType

    assert P * sum(G_LIST) == B_LOC
    FD = G_MAX * SEQ    # max data cols in a/b/d tiles
    FS = G_MAX * GS + 1 # max cols in s/w tiles

    nc = bacc.Bacc("TRN2", target_bir_lowering=False, debug=False)
    a_ext = nc.declare_dram_parameter("a", [B_LOC, SEQ], F32, isOutput=False)
    b_ext = nc.declare_dram_parameter("b", [B_LOC, SEQ], F32, isOutput=False)
    eye_ext = nc.declare_dram_parameter("eye", [P, P], F32, isOutput=False)
    o_ext = nc.declare_dram_parameter("out", [B_LOC, SEQ], F32, isOutput=True)


    with tile.TileContext(nc) as tc, ExitStack() as ctx:
        cpool = ctx.enter_context(tc.tile_pool(name="const", bufs=1))
        ten = cpool.tile([P, FS], F32)
        nc.vector.memset(ten[:], 10.0)
        eye_t = cpool.tile([P, P], F32)
        nc.sync.dma_start(out=eye_t[:], in_=eye_ext[:])
        # persistent ping-pong s tiles; separator cols written once
        s_pp = [cpool.tile([P, FS], F32, tag=f"s{i}", name=f"s_pp{i}")
                for i in range(N_SPP)]
        for s_t in s_pp:
            nc.vector.memset(s_t[:, 0:FS:GS], 0.0)

        io = ctx.enter_context(tc.tile_pool(name="io", bufs=IO_BUFS))
        wk = ctx.enter_context(tc.tile_pool(name="wk", bufs=WK_BUFS))
        ps = ctx.enter_context(tc.tile_pool(name="ps", bufs=8, space="PSUM"))

        base = 0
        for t, Gt in enumerate(G_LIST):
            FDt = Gt * SEQ
            FSt = Gt * GS + 1
            mmn = min(MMN, FDt)
            n_mm = FDt // mmn
            gpc = mmn // SEQ
            a_vt = a_ext[:][base:base + P * Gt].rearrange(
                "(p g) e -> p (g e)", p=P)
            b_vt = b_ext[:][base:base + P * Gt].rearrange(
                "(p g) e -> p (g e)", p=P)
            o_vt = o_ext[:][base:base + P * Gt].rearrange(
                "(p g) e -> p (g e)", p=P)
            base += P * Gt

            a_t = io.tile([P, FDt], F32, tag="a", name=f"a_{t}",
                          padded_shape=[P, FD])
            b_t = io.tile([P, FDt], F32, tag="b", name=f"b_{t}",
                          padded_shape=[P, FD])
            nc.sync.dma_start(out=a_t[:], in_=a_vt)
            nc.sync.dma_start(out=b_t[:], in_=b_vt)

            # s = a + b on PE; ACT drains each PSUM bank into the
            # LSB-first separator layout (reversal in the access pattern)
            s_full = s_pp[t % N_SPP]
            for j in range(n_mm):
                ps_j = ps.tile([P, mmn], F32, tag="ps", name=f"ps_{t}_{j}")
                cols = bass.ts(j, mmn)
                nc.tensor.matmul(ps_j[:], eye_t[:], a_t[:, cols],
                                 start=True, stop=False)
                nc.tensor.matmul(ps_j[:], eye_t[:], b_t[:, cols],
                                 start=False, stop=True)
                ps_rev = ps_j[:].rearrange("p (g e) -> p g e",
                                           e=SEQ)[:, :, ::-1]
                s_dj = s_full[:, 1 + j * gpc * GS:].rearrange(
                    "p (g e) -> p g e", e=GS)[:, 0:gpc, 0:SEQ]
                nc.scalar.activation(s_dj, ps_rev, ACTF.Copy)

            # v_t = [10 <= v_{t-1}] + s_t : the whole carry chain
            w_t = wk.tile([P, FSt], F32, tag="w", name=f"w_{t}",
                          padded_shape=[P, FS])
            nc.vector.tensor_tensor_scan(
                out=w_t[:], data0=ten[:, 0:FSt], data1=s_full[:, 0:FSt],
                initial=0.0, op0=ALU.is_le, op1=ALU.add)

            # c = [v >= 10] staged in d (2x mode), then
            # digit = c*(-10) + v in place, written MSB-first
            d_t = wk.tile([P, FDt], F32, tag="d", name=f"d_{t}",
                          padded_shape=[P, FD])
            w_data = w_t[:, 1:].rearrange("p (g e) -> p g e",
                                          e=GS)[:, :, 0:SEQ]
            d_rev = d_t[:].rearrange("p (g e) -> p g e", e=SEQ)[:, :, ::-1]
            nc.vector.tensor_scalar(out=d_rev, in0=w_data, scalar1=10.0,
                                    scalar2=None, op0=ALU.is_ge)
            nc.vector.scalar_tensor_tensor(
                out=d_rev, in0=d_rev, scalar=-10.0, in1=w_data,
                op0=ALU.mult, op1=ALU.add)

            nc.scalar.dma_start(out=o_vt, in_=d_t[:])

    nc.finalize()
    return nc


def kernel(a, b, weight_ih=None, weight_hh=None, bias_ih=None, bias_hh=None):
    """Full-batch digit adder. The RNN weights are the fixed carry-add
    weights baked into the module; the kernel implements that function
    directly, so they are accepted and unused."""
    from concourse.bass_utils import run_bass_kernel_spmd

    a = np.ascontiguousarray(np.asarray(a, dtype=np.float32))
    b = np.ascontiguousarray(np.asarray(b, dtype=np.float32))
    assert a.shape == (BATCH, SEQ) and b.shape == (BATCH, SEQ)

    if "nc" not in _nc_cache:
        _nc_cache["nc"] = _build_adder()
    nc = _nc_cache["nc"]

    eye = np.eye(P, dtype=np.float32)
    in_maps = [
        {"a": a[i * B_LOC:(i + 1) * B_LOC],
         "b": b[i * B_LOC:(i + 1) * B_LOC],
         "eye": eye}
        for i in range(N_CORES)
    ]
    res = run_bass_kernel_spmd(nc, in_maps, core_ids=list(range(N_CORES)))
    return np.concatenate(
        [res.results[i]["out"] for i in range(N_CORES)], axis=0)


if __name__ == "__main__":
    rng = np.random.default_rng(0)
    a = rng.integers(0, 10, (BATCH, SEQ)).astype(np.float32)
    b = rng.integers(0, 10, (BATCH, SEQ)).astype(np.float32)
    out = kernel(a, b)
    # host reference
    c = np.zeros(BATCH, np.float32)
    exp = np.zeros_like(a)
    for e in range(SEQ - 1, -1, -1):
        s = a[:, e] + b[:, e] + c
        c = (s >= 10).astype(np.float32)
        exp[:, e] = s - 10 * c
    print("max abs err:", np.abs(out - exp).max())



# revision 5
# speedup vs baseline: 2.7440x; 2.7440x over previous
"""Trainium2 Bass kernel: batched 64-digit base-10 addition (nn_Adder).

The reference RNN scan is carry-propagating decimal addition. This kernel
works in radix-100: each pair of digits is one limb, halving the length of
the sequential carry chain the DVE has to walk.

Per core (pure data parallel across 8 cores, batch 524288 -> 65536 rows):

  * Inputs are uploaded as fp8e4 (digits 0-9 are exact in e4m3), cutting
    input HBM traffic 4x vs f32. Output leaves the device as one uint8 per
    digit PAIR (the base-100 value 10*d_hi + d_lo), cutting output traffic
    8x; the host splits pairs back into digit columns with divmod.
  * PE forms limb sums straight from the fp8 digit tiles: per 512-col PSUM
    window, 4 accumulating matmuls with scaled identity weights
    (10*I @ a_hi, I @ a_lo, 10*I @ b_hi, I @ b_lo), where a_hi/a_lo are
    stride -2 views that also fold in the MSB->LSB reversal. PSUM then
    holds s[m] = 10*(a+b)_hi + (a+b)_lo in LSB-first limb order.
  * DVE runs the carry chain in ONE scan instruction per tile, reading
    PSUM directly:  v_t = [100 <= v_{t-1}] + s_t.  Row boundaries are
    handled by the data0 operand: a constant pattern tile holding 100.0
    everywhere except 3e9 at each row's first limb, so [3e9 <= v] = 0
    kills the carry between consecutive rows packed in one partition.
  * The scan state v = pair + 100*carry is <= 199, so the scan writes
    the OUTPUT tile directly as uint8 (exact downcast) - no extraction
    instructions at all. The host decodes raw%100 -> divmod 10 and
    flips the limb order back to MSB-first (a numpy view).
  * ACT only issues output DMAs; GpSimd untouched (it would steal the
    DVE SBUF ports).

All intermediate values are small integers (scan state <= 199), exact in
f32/u8 -> bit-exact output after the host-side decode.
"""

import sys

sys.path.insert(0, "/opt/trn_rl_repo")

import numpy as np

BATCH = 524288
SEQ = 64
N_CORES = 8
B_LOC = BATCH // N_CORES

P = 128
LIMBS = SEQ // 2    # 32 base-100 limbs per row
G = 64              # rows packed per partition per tile
T = (B_LOC // P) // G   # 8 tiles
ROWS_T = P * G      # 8192 rows per tile
FD = G * SEQ        # 4096 digit cols per partition (a/b tiles)
FT = G * LIMBS      # 2048 limb cols per partition (psum/w/d tiles)
MW = 512            # matmul window = one PSUM bank of f32

IO_BUFS = 3
WK_BUFS = 3

_nc_cache = {}


def _build_adder():
    from contextlib import ExitStack

    import concourse.bacc as bacc
    import concourse.mybir as mybir
    import concourse.tile as tile

    F32 = mybir.dt.float32
    F16 = mybir.dt.float16
    FP8 = mybir.dt.float8e4
    U8 = mybir.dt.uint8
    ALU = mybir.AluOpType

    nc = bacc.Bacc("TRN2", target_bir_lowering=False, debug=False)
    a_ext = nc.declare_dram_parameter("a", [B_LOC, SEQ], FP8, isOutput=False)
    b_ext = nc.declare_dram_parameter("b", [B_LOC, SEQ], FP8, isOutput=False)
    e10_ext = nc.declare_dram_parameter("eye10", [P, P], FP8, isOutput=False)
    e1_ext = nc.declare_dram_parameter("eye1", [P, P], FP8, isOutput=False)
    o_ext = nc.declare_dram_parameter("out", [B_LOC, LIMBS], U8, isOutput=True)

    with tile.TileContext(nc) as tc, ExitStack() as ctx:
        cpool = ctx.enter_context(tc.tile_pool(name="const", bufs=1))
        # scan data0: 100.0 everywhere, "+inf" at each row's first limb so
        # the carry chain resets at row boundaries within a partition
        pat = cpool.tile([P, FT], F32)
        nc.vector.memset(pat[:], 100.0)
        nc.vector.memset(pat[:, 0:FT:LIMBS], 3.0e9)
        e10 = cpool.tile([P, P], FP8)
        e1 = cpool.tile([P, P], FP8)
        nc.sync.dma_start(out=e10[:], in_=e10_ext[:])
        nc.sync.dma_start(out=e1[:], in_=e1_ext[:])

        io = ctx.enter_context(tc.tile_pool(name="io", bufs=IO_BUFS))
        wk = ctx.enter_context(tc.tile_pool(name="wk", bufs=WK_BUFS))
        ps = ctx.enter_context(tc.tile_pool(name="ps", bufs=2, space="PSUM"))

        RW = MW // LIMBS    # rows covered by one matmul window
        for t in range(T):
            rows = slice(t * ROWS_T, (t + 1) * ROWS_T)
            a_vt = a_ext[:][rows].rearrange("(p g) e -> p (g e)", p=P)
            b_vt = b_ext[:][rows].rearrange("(p g) e -> p (g e)", p=P)
            o_vt = o_ext[:][rows].rearrange("(p g) e -> p (g e)", p=P)

            a_t = io.tile([P, FD], FP8, tag="a", name=f"a_{t}")
            b_t = io.tile([P, FD], FP8, tag="b", name=f"b_{t}")
            nc.sync.dma_start(out=a_t[:], in_=a_vt)
            nc.sync.dma_start(out=b_t[:], in_=b_vt)

            # limb sums on PE: per window, 4 accumulating matmuls with
            # stride -2 digit views (reversal folded into the AP)
            ps_t = ps.tile([P, FT], F32, tag="ps", name=f"ps_{t}")
            A3 = a_t[:].rearrange("p (r e) -> p r e", e=SEQ)
            B3 = b_t[:].rearrange("p (r e) -> p r e", e=SEQ)
            for h in range(FT // MW):
                win = ps_t[:, h * MW:(h + 1) * MW]
                rs = slice(h * RW, (h + 1) * RW)
                nc.tensor.matmul(win, e10[:], A3[:, rs, 62::-2],
                                 start=True, stop=False)
                nc.tensor.matmul(win, e1[:], A3[:, rs, 63::-2],
                                 start=False, stop=False)
                nc.tensor.matmul(win, e10[:], B3[:, rs, 62::-2],
                                 start=False, stop=False)
                nc.tensor.matmul(win, e1[:], B3[:, rs, 63::-2],
                                 start=False, stop=True)

            # whole carry chain: v_t = [100 <= v_{t-1}] + s_t, written
            # straight to the u8 output tile (v <= 199, exact in u8)
            d_t = wk.tile([P, FT], U8, tag="d", name=f"d_{t}")
            nc.vector.tensor_tensor_scan(
                out=d_t[:], data0=pat[:], data1=ps_t[:],
                initial=0.0, op0=ALU.is_le, op1=ALU.add)

            nc.scalar.dma_start(out=o_vt, in_=d_t[:])

    nc.finalize()
    return nc


def _host_inputs(a, b):
    """Cast digit arrays to fp8 (exact for 0..9) and build per-core maps."""
    import ml_dtypes

    fp8 = ml_dtypes.float8_e4m3
    a8 = np.ascontiguousarray(np.asarray(a, dtype=np.float32)).astype(fp8)
    b8 = np.ascontiguousarray(np.asarray(b, dtype=np.float32)).astype(fp8)
    eye10 = (10.0 * np.eye(P, dtype=np.float32)).astype(fp8)
    eye1 = np.eye(P, dtype=np.float32).astype(fp8)
    return [
        {"a": a8[i * B_LOC:(i + 1) * B_LOC],
         "b": b8[i * B_LOC:(i + 1) * B_LOC],
         "eye10": eye10, "eye1": eye1}
        for i in range(N_CORES)
    ]


def _host_decode(results):
    """Concat per-core raw scan bytes (v = pair + 100*carry, LSB-first
    limb order) and decode into f32 digit columns."""
    raw = np.concatenate(
        [results[i]["out"] for i in range(N_CORES)], axis=0)  # (B, 32) u8
    pairs = raw[:, ::-1] % 100
    hi, lo = np.divmod(pairs, 10)
    out = np.empty((BATCH, SEQ), dtype=np.float32)
    out[:, 0::2] = hi
    out[:, 1::2] = lo
    return out


def kernel(a, b, weight_ih=None, weight_hh=None, bias_ih=None, bias_hh=None):
    """Full-batch digit adder. The RNN weights are the fixed carry-add
    weights baked into the module; the kernel implements that function
    directly, so they are accepted and unused."""
    from concourse.bass_utils import run_bass_kernel_spmd

    assert np.asarray(a).shape == (BATCH, SEQ)
    assert np.asarray(b).shape == (BATCH, SEQ)

    if "nc" not in _nc_cache:
        _nc_cache["nc"] = _build_adder()
    nc = _nc_cache["nc"]

    res = run_bass_kernel_spmd(nc, _host_inputs(a, b),
                               core_ids=list(range(N_CORES)))
    return _host_decode(res.results)


if __name__ == "__main__":
    rng = np.random.default_rng(0)
    a = rng.integers(0, 10, (BATCH, SEQ)).astype(np.float32)
    b = rng.integers(0, 10, (BATCH, SEQ)).astype(np.float32)
    out = kernel(a, b)
    # host reference
    c = np.zeros(BATCH, np.float32)
    exp = np.zeros_like(a)
    for e in range(SEQ - 1, -1, -1):
        s = a[:, e] + b[:, e] + c
        c = (s >= 10).astype(np.float32)
        exp[:, e] = s - 10 * c
    print("max abs err:", np.abs(out - exp).max())


# revision 8
# speedup vs baseline: 2.9565x; 1.0775x over previous
"""Trainium2 Bass kernel: batched 64-digit base-10 addition (nn_Adder).

The reference RNN scan is carry-propagating decimal addition. The DVE
scan instruction is the only engine that can walk the carry recurrence,
and it runs at a fixed ~2.15 ns/element regardless of dtype — so the
kernel works in radix-10^4: each group of FOUR digits is one limb,
quartering the scan length per row (16 limbs instead of 64 digits).

Per core (pure data parallel across 8 cores, batch 524288 -> 65536 rows):

  * Inputs are uploaded as fp8e4 (digits 0-9 exact in e4m3): 4x less
    input HBM traffic than f32. Output leaves as one uint16 per FOUR
    digits (the raw scan state v = limb + 10^4*carry <= 19999, exact in
    u16): 8x less output traffic. The host decodes v % 10^4 into digit
    columns with numpy divmods.
  * Stage 1 (PE, fp8 DoubleRow perf mode, 0.5 cyc/row): one DoubleRow
    matmul per source computes 10*d_even + d_odd for every digit pair —
    the weights [10I | I] pair with strided views of the even/odd digit
    positions (k-tile dim), and the MSB->LSB reversal is folded into the
    same access pattern. PSUM gets base-100 limbs M <= 198, LSB-first.
  * ACT drains M to SBUF as bf16 (integers <= 256 exact in bf16).
  * Stage 2 (PE, bf16): two accumulating matmuls with weights 100I / I
    over the odd/even base-100 limbs -> PSUM radix-10^4 limbs <= 19998.
  * DVE runs the whole carry chain in ONE scan per tile, reading PSUM:
    v_t = [10^4 <= v_{t-1}] + s_t. Row boundaries are killed by the
    data0 pattern operand (3e9 at each row's first limb). The scan
    writes the u16 OUTPUT tile directly - zero post-processing.
  * GpSimd untouched (it would steal the DVE SBUF ports).

All intermediate values are small integers, exact in fp8/bf16/f32/u16 ->
bit-exact output after the host-side decode.
"""

import sys

sys.path.insert(0, "/opt/trn_rl_repo")

import numpy as np

BATCH = 524288
SEQ = 64
N_CORES = 8
B_LOC = BATCH // N_CORES

P = 128
LIMBS2 = SEQ // 2   # 32 base-100 limbs per row (stage-1)
LIMBS4 = SEQ // 4   # 16 base-10^4 limbs per row (stage-2 / scan / output)
G = 32              # rows packed per partition per tile
T = (B_LOC // P) // G   # 16 tiles
ROWS_T = P * G      # 4096 rows per tile
FD = G * SEQ        # 2048 digit cols per partition (a/b tiles)
F2 = G * LIMBS2     # 1024 base-100 limb cols per partition
F4 = G * LIMBS4     # 512 base-10^4 limb cols per partition
MW = 512            # matmul window = one PSUM bank of f32

IO_BUFS = T         # all input DMAs queued up front
WK_BUFS = 3

_nc_cache = {}


def _build_adder():
    from contextlib import ExitStack

    import concourse.bacc as bacc
    import concourse.mybir as mybir
    import concourse.tile as tile

    F32 = mybir.dt.float32
    BF16 = mybir.dt.bfloat16
    FP8 = mybir.dt.float8e4
    U16 = mybir.dt.uint16
    ALU = mybir.AluOpType
    DR = mybir.MatmulPerfMode.DoubleRow

    nc = bacc.Bacc("TRN2", target_bir_lowering=False, debug=False)
    a_ext = nc.declare_dram_parameter("a", [B_LOC, SEQ], FP8, isOutput=False)
    b_ext = nc.declare_dram_parameter("b", [B_LOC, SEQ], FP8, isOutput=False)
    # [10I | I] fp8 pair-weights for DoubleRow stage 1
    edr_ext = nc.declare_dram_parameter("eyedr", [P, 2 * P], FP8,
                                        isOutput=False)
    # 100I and I in bf16 for stage 2
    e100_ext = nc.declare_dram_parameter("eye100", [P, P], BF16,
                                         isOutput=False)
    e1_ext = nc.declare_dram_parameter("eye1", [P, P], BF16, isOutput=False)
    o_ext = nc.declare_dram_parameter("out", [B_LOC, LIMBS4], U16,
                                      isOutput=True)

    with tile.TileContext(nc) as tc, ExitStack() as ctx:
        cpool = ctx.enter_context(tc.tile_pool(name="const", bufs=1))
        # scan data0: 10^4 everywhere, "+inf" at each row's first limb so
        # the carry chain resets at row boundaries within a partition
        pat = cpool.tile([P, F4], F32)
        nc.vector.memset(pat[:], 10000.0)
        nc.vector.memset(pat[:, 0:F4:LIMBS4], 3.0e9)
        edr = cpool.tile([P, 2 * P], FP8)
        e100 = cpool.tile([P, P], BF16)
        e1 = cpool.tile([P, P], BF16)
        nc.sync.dma_start(out=edr[:], in_=edr_ext[:])
        nc.sync.dma_start(out=e100[:], in_=e100_ext[:])
        nc.sync.dma_start(out=e1[:], in_=e1_ext[:])

        io = ctx.enter_context(tc.tile_pool(name="io", bufs=IO_BUFS))
        wk = ctx.enter_context(tc.tile_pool(name="wk", bufs=WK_BUFS))
        ps1 = ctx.enter_context(tc.tile_pool(name="ps1", bufs=2,
                                             space="PSUM"))
        ps2 = ctx.enter_context(tc.tile_pool(name="ps2", bufs=2,
                                             space="PSUM"))

        edr3 = edr[:].rearrange("p (t m) -> p t m", t=2)
        RW1 = MW // LIMBS2      # rows per stage-1 window (16)
        RW2 = MW // LIMBS4      # rows per stage-2 window (32)

        for t in range(T):
            rows = slice(t * ROWS_T, (t + 1) * ROWS_T)
            a_vt = a_ext[:][rows].rearrange("(p g) e -> p (g e)", p=P)
            b_vt = b_ext[:][rows].rearrange("(p g) e -> p (g e)", p=P)
            o_vt = o_ext[:][rows].rearrange("(p g) e -> p (g e)", p=P)

            a_t = io.tile([P, FD], FP8, tag="a", name=f"a_{t}")
            b_t = io.tile([P, FD], FP8, tag="b", name=f"b_{t}")
            nc.sync.dma_start(out=a_t[:], in_=a_vt)
            nc.sync.dma_start(out=b_t[:], in_=b_vt)

            # stage 1: base-100 limbs M = 10*(a+b)_hi + (a+b)_lo on PE.
            # rhs AP dims [p, t(k-tile), row, limb]: t picks the hi/lo
            # digit of each pair, limb stride -2 folds in the reversal.
            ps_t = ps1.tile([P, F2], F32, tag="ps1", name=f"ps1_{t}")
            A4 = a_t[:].rearrange("p (r m2 t) -> p t r m2",
                                  t=2, m2=LIMBS2)[:, :, :, ::-1]
            B4 = b_t[:].rearrange("p (r m2 t) -> p t r m2",
                                  t=2, m2=LIMBS2)[:, :, :, ::-1]
            for h in range(F2 // MW):
                win = ps_t[:, h * MW:(h + 1) * MW]
                rs = slice(h * RW1, (h + 1) * RW1)
                nc.tensor.matmul(win, edr3, A4[:, :, rs], start=True,
                                 stop=False, perf_mode=DR)
                nc.tensor.matmul(win, edr3, B4[:, :, rs], start=False,
                                 stop=True, perf_mode=DR)

            # ACT drains M to SBUF bf16 (exact, M <= 198)
            m_t = wk.tile([P, F2], BF16, tag="m", name=f"m_{t}")
            nc.scalar.activation(m_t[:], ps_t[:],
                                 mybir.ActivationFunctionType.Copy)

            # stage 2: radix-10^4 limbs L = 100*M_odd + M_even on PE
            ps4_t = ps2.tile([P, F4], F32, tag="ps2", name=f"ps2_{t}")
            M3 = m_t[:].rearrange("p (r q t) -> p r q t", t=2, q=LIMBS4)
            for h in range(F4 // MW):
                win = ps4_t[:, h * MW:(h + 1) * MW]
                rs = slice(h * RW2, (h + 1) * RW2)
                nc.tensor.matmul(win, e100[:], M3[:, rs, :, 1], start=True,
                                 stop=False)
                nc.tensor.matmul(win, e1[:], M3[:, rs, :, 0], start=False,
                                 stop=True)

            # whole carry chain: v_t = [10^4 <= v_{t-1}] + s_t, written
            # straight to the u16 output tile (v <= 19999, exact)
            d_t = wk.tile([P, F4], U16, tag="d", name=f"d_{t}")
            nc.vector.tensor_tensor_scan(
                out=d_t[:], data0=pat[:], data1=ps4_t[:],
                initial=0.0, op0=ALU.is_le, op1=ALU.add)

            nc.scalar.dma_start(out=o_vt, in_=d_t[:])

    nc.finalize()
    return nc


def _host_inputs(a, b):
    """Cast digit arrays to fp8 (exact for 0..9) and build per-core maps."""
    import ml_dtypes

    fp8 = ml_dtypes.float8_e4m3
    bf16 = ml_dtypes.bfloat16
    a8 = np.ascontiguousarray(np.asarray(a, dtype=np.float32)).astype(fp8)
    b8 = np.ascontiguousarray(np.asarray(b, dtype=np.float32)).astype(fp8)
    eye = np.eye(P, dtype=np.float32)
    eyedr = np.concatenate([10.0 * eye, eye], axis=1).astype(fp8)
    eye100 = (100.0 * eye).astype(bf16)
    eye1 = eye.astype(bf16)
    return [
        {"a": a8[i * B_LOC:(i + 1) * B_LOC],
         "b": b8[i * B_LOC:(i + 1) * B_LOC],
         "eyedr": eyedr, "eye100": eye100, "eye1": eye1}
        for i in range(N_CORES)
    ]


def _host_decode(results):
    """Concat per-core raw scan words (v = limb + 10^4*carry, LSB-first
    limb order) and decode into f32 digit columns."""
    raw = np.concatenate(
        [results[i]["out"] for i in range(N_CORES)], axis=0)  # (B, 16) u16
    v = (raw[:, ::-1] % 10000).astype(np.int32)
    out = np.empty((BATCH, SEQ), dtype=np.float32)
    q, out_3 = np.divmod(v, 10)
    q, out_2 = np.divmod(q, 10)
    out_0, out_1 = np.divmod(q, 10)
    out[:, 0::4] = out_0
    out[:, 1::4] = out_1
    out[:, 2::4] = out_2
    out[:, 3::4] = out_3
    return out


def kernel(a, b, weight_ih=None, weight_hh=None, bias_ih=None, bias_hh=None):
    """Full-batch digit adder. The RNN weights are the fixed carry-add
    weights baked into the module; the kernel implements that function
    directly, so they are accepted and unused."""
    from concourse.bass_utils import run_bass_kernel_spmd

    assert np.asarray(a).shape == (BATCH, SEQ)
    assert np.asarray(b).shape == (BATCH, SEQ)

    if "nc" not in _nc_cache:
        _nc_cache["nc"] = _build_adder()
    nc = _nc_cache["nc"]

    res = run_bass_kernel_spmd(nc, _host_inputs(a, b),
                               core_ids=list(range(N_CORES)))
    return _host_decode(res.results)


if __name__ == "__main__":
    rng = np.random.default_rng(0)
    a = rng.integers(0, 10, (BATCH, SEQ)).astype(np.float32)
    b = rng.integers(0, 10, (BATCH, SEQ)).astype(np.float32)
    out = kernel(a, b)
    # host reference
    c = np.zeros(BATCH, np.float32)
    exp = np.zeros_like(a)
    for e in range(SEQ - 1, -1, -1):
        s = a[:, e] + b[:, e] + c
        c = (s >= 10).astype(np.float32)
        exp[:, e] = s - 10 * c
    print("max abs err:", np.abs(out - exp).max())
